# revision 40
# baseline (speedup 1.0000x reference)
"""Longformer sliding-window + global attention layer on 8 Trainium2 NeuronCores.

Sharding: sequence-parallel over the 4096 tokens (512 per core, all 12 heads).
Each core recomputes the k/v halo (256 tokens each side) and the 64 global
k/v tokens locally from zero-padded hsT input, so the program is uniform SPMD.
The global-query rows (first 64 tokens attend to everything) are computed as
flash-style partial sums over each core's 512 tokens, AllReduced directly into
an output DRAM buffer, and normalized on the host — no on-device consumer of
the collective exists, so it can never stall the banded pipeline.

v3 layout strategy (all matmuls fp16, PSUM accumulate f32). The PE runs
~0.55 ns/row only when both the contraction and output-partition dims are a
full 128; 64-wide geometry costs ~0.9-1.05 ns/row. So:
  - qT / qgT are stored zero-padded per head (12 chunks of [128, t]: head h
    occupies feature rows 64*(h%2)..+64, zeros elsewhere) so every score
    matmul contracts over the full 128 partitions of the shared k chunk —
    the other head's k columns are killed by the zeros.
  - vE uses 128-wide head blocks (64 v | ones | 63 zeros) so PV matmuls have
    128 output partitions; the softmax denominator lands in PSUM partition 64
    via the ones column. exg (global-column probs) is kept in persistent
    [128, 512] tiles whose lower 64 rows are zeroed once, so the global-column
    PV also contracts over 128.
  - Banded attention uses transposed scores over the core's 8 extended-window
    key tiles (jx) in pairs sharing a 2-bank PSUM tile (one exp per pair);
    0/1 mask multiplies cover only band/col boundary regions. A PE transpose
    + per-partition reciprocal scale normalizes [t, d] tiles.
"""
import numpy as np

import concourse.bacc as bacc
import concourse.mybir as mybir
import concourse.tile as tile
from concourse.bass_utils import run_bass_kernel_spmd

F32 = mybir.dt.float32
F16 = mybir.dt.float16
Exp = mybir.ActivationFunctionType.Exp

S, H, NH, HD = 4096, 768, 12, 64
C = 256               # chunk / one-sided window
G = 64                # global tokens
NCORE = 8
TPC = S // NCORE      # 512 tokens per core
EXT = TPC + 2 * C     # 1024 ext window
COLS = EXT + G        # 1088 = ext | glob
KC = H // 128         # 6 hidden chunks
VW = 66               # per-head vgN block: 64 v | ones | pad
EW = 128              # per-head vE block: 64 v | ones | 63 zeros
SCALE = 1.0 / 8.0     # 1/sqrt(HD)

# per key-tile jx: (t0, tn) query range its chunk windows cover, and the
# t-range needing the 0/1 mask multiply
JX_T = {0: (0, 256), 1: (0, 256), 2: (0, 512), 3: (0, 512),
        4: (0, 512), 5: (0, 512), 6: (256, 512), 7: (256, 512)}
JX_MASK = {0: (0, 256), 1: (0, 256), 2: (0, 512), 3: (256, 512),
           4: (0, 256), 5: (0, 256), 6: (256, 512), 7: (256, 512)}
# pairs share one [128, 1024] PSUM tile; (2,3) first so PV start=True is full
JX_PAIRS = [(2, 3), (4, 5), (0, 1), (6, 7)]
# hsT/kT columns are stored [own 512 | haloL 256 | haloR 256 | glob 64] so the
# startup-critical own+glob pieces can load (and project) before the halos;
# these map an ext-order key tile jx / v tile tt to its stored column
JX_COL = {2: 0, 3: 128, 4: 256, 5: 384, 0: 512, 1: 640, 6: 768, 7: 896}
TT_COL = {0: 512, 1: 640, 2: 0, 3: 128, 4: 256, 5: 384, 6: 768, 7: 896,
          8: 1024}
# packed col offset of each jx's mask region in the [128, 2304] masks input
JX_PACK = {2: 0, 3: 512, 4: 768, 5: 1024, 0: 1280, 1: 1536, 6: 1792, 7: 2048}
# mask ops: (pair_index, ex-tile col range, packed range, engine)
MASK_OPS = [
    (0, (0, 512), (0, 512), "v"),         # jx2
    (0, (768, 1024), (512, 768), "v"),    # jx3
    (1, (0, 256), (768, 1024), "v"),      # jx4
    (1, (512, 768), (1024, 1280), "v"),   # jx5
    (2, (0, 512), (1280, 1792), "v"),     # jx0|jx1 contiguous
    (3, (0, 512), (1792, 2304), "v"),     # jx6|jx7 contiguous
]

_PROG_CACHE = {}


def _build_program(with_bias: bool):
    nc = bacc.Bacc("TRN2", target_bir_lowering=False, debug=False,
                   num_devices=NCORE)
    d_hsT = nc.declare_dram_parameter("hsT", [128, KC * COLS], F16,
                                      isOutput=False)
    d_w = {
        n: nc.declare_dram_parameter(n, [128, KC * H], F16, isOutput=False)
        for n in ("wq", "wk", "wv", "wkg", "wvg", "wqg")
    }
    d_masks = nc.declare_dram_parameter("masks", [128, 2304], F16,
                                        isOutput=False)
    CW = 128 + NH * EW + NH * VW    # ident | vE ones image | vgN ones image
    d_consts = nc.declare_dram_parameter("consts", [128, CW], F16,
                                         isOutput=False)
    if with_bias:
        d_brow = nc.declare_dram_parameter("biasrow", [7, COLS], F16,
                                           isOutput=False)
    d_out = nc.declare_dram_parameter("out", [TPC, H], F16, isOutput=True)
    d_outgraw = nc.declare_dram_parameter("outgraw", [G, NH * VW], F32,
                                          isOutput=True)

    with tile.TileContext(nc) as tc:
        with (
            tc.tile_pool(name="const", bufs=1) as const,
            tc.tile_pool(name="work", bufs=2) as work,
            tc.tile_pool(name="w2", bufs=3) as w2,
            tc.tile_pool(name="late", bufs=1) as late,
            tc.tile_pool(name="dram", bufs=2, space="DRAM") as dram,
            tc.tile_pool(name="psQ", bufs=2, space="PSUM") as psQ,
            tc.tile_pool(name="psO", bufs=2, space="PSUM") as psO,
            tc.tile_pool(name="psT", bufs=2, space="PSUM") as psT,
        ):
            engs = [nc.sync, nc.scalar, nc.gpsimd]
            eng_i = [0]

            def next_eng():
                e = engs[eng_i[0] % 3]
                eng_i[0] += 1
                return e

            # ---- resident loads, chunked + ordered by first consumer; the
            # startup-critical own-token and glob pieces come first, the halo
            # pieces (only needed by kT/vE) stream in during the glob phase
            hsb_t = late.tile([128, KC, COLS], F16, tag="ph")
            for kc in range(KC):
                next_eng().dma_start(
                    out=hsb_t[:, kc, 0:512],
                    in_=d_hsT[:, kc * COLS:kc * COLS + 512])
            for kc in range(KC):
                next_eng().dma_start(
                    out=hsb_t[:, kc, 1024:1088],
                    in_=d_hsT[:, kc * COLS + 1024:kc * COLS + 1088])
            hsb = [hsb_t[:, kc, :] for kc in range(KC)]

            def load_w(name):
                t = const.tile([128, KC, H], F16, tag=f"w_{name}",
                               name=f"w_{name}")
                for kc in range(KC):
                    next_eng().dma_start(
                        out=t[:, kc, :], in_=d_w[name][:, kc * H:(kc + 1) * H])
                return [t[:, kc, :] for kc in range(KC)]

            wvg_t = load_w("wvg")
            csb = const.tile([128, CW], F16)
            next_eng().dma_start(out=csb, in_=d_consts[:])
            ident = csb[:, 0:128]
            # per EW-block: col 64 = 1.0 (ones col), rest 0
            vones_e = csb[:, 128:128 + NH * EW].rearrange(
                "p (h x) -> p h x", x=EW)
            # per VW-block: col 64 = 1.0, col 65 = 0
            vones_g = csb[:, 128 + NH * EW:CW].rearrange(
                "p (h x) -> p h x", x=VW)
            if with_bias:
                bsb = const.tile([7, COLS], F16)
                next_eng().dma_start(out=bsb, in_=d_brow[:])
            wqg_t = load_w("wqg")
            wkg_t = load_w("wkg")
            for kc in range(KC):
                next_eng().dma_start(
                    out=hsb_t[:, kc, 512:1024],
                    in_=d_hsT[:, kc * COLS + 512:kc * COLS + 1024])
            wq_t = load_w("wq")
            msb = const.tile([128, 2304], F16)
            next_eng().dma_start(out=msb, in_=d_masks[:])
            wk_t = load_w("wk")
            wv_t = load_w("wv")

            kT = const.tile([128, KC, COLS], F16)    # [o, t] all heads
            qT = const.tile([128, NH, TPC], F16)     # zero-padded per head
            vE = const.tile([128, 9, NH * EW], F16)  # natural v + ones cols
            kgT = const.tile([128, KC, TPC], F16)
            vgN = const.tile([128, 4, NH * VW], F16)
            qgT = const.tile([128, NH, G], F16)      # zero-padded per head
            exg2 = [const.tile([128, TPC], F16, tag=f"exg{i}",
                               name=f"exg{i}")
                    for i in range(2)]
            # zero the pad regions once, while the PE waits on input DMA
            nc.vector.memset(qT[:, :, :], 0.0)
            nc.vector.memset(qgT[:, :, :], 0.0)
            nc.vector.memset(vE[64:128, 8, :], 0.0)
            nc.vector.memset(exg2[0][:, :], 0.0)
            nc.vector.memset(exg2[1][:, :], 0.0)

            def proj_T(dst, wsb, segs, bias_idx, dst_off, split=False,
                       hook=None):
                # dst[o, t] = W.T @ hsT cols; split=True scatters the two
                # 64-row head halves into zero-padded per-head chunks
                for oc in range(KC):
                    for c0, cn in segs:
                        ps = psQ.tile([128, 512], F32, tag="psQ")
                        for kc in range(KC):
                            nc.tensor.matmul(
                                out=ps[:, 0:cn],
                                lhsT=wsb[kc][:, oc * 128:(oc + 1) * 128],
                                rhs=hsb[kc][:, c0:c0 + cn],
                                start=(kc == 0),
                                stop=(kc == KC - 1 and not with_bias),
                            )
                        if with_bias:
                            nc.tensor.matmul(
                                out=ps[:, 0:cn],
                                lhsT=bsb[1 + bias_idx:2 + bias_idx,
                                         oc * 128:(oc + 1) * 128],
                                rhs=bsb[0:1, 0:cn],
                                start=False, stop=True,
                            )
                        d0 = c0 - dst_off
                        if split:
                            nc.vector.tensor_copy(
                                out=dst[0:64, 2 * oc, d0:d0 + cn],
                                in_=ps[0:64, 0:cn])
                            nc.vector.tensor_copy(
                                out=dst[64:128, 2 * oc + 1, d0:d0 + cn],
                                in_=ps[64:128, 0:cn])
                        else:
                            nc.vector.tensor_copy(
                                out=dst[:, oc, d0:d0 + cn], in_=ps[:, 0:cn])
                    if hook is not None:
                        hook(oc)

            def proj_nat(dst, wsb, tts, bias_idx, bw, vones):
                # dst[t, head-block] with bw-stride head blocks
                for ti, tt in enumerate(tts):
                    # only the ones|pad columns of each block need init;
                    # value cols are overwritten by the PSUM evacuation
                    nc.vector.tensor_copy(
                        out=dst[:, ti, :].rearrange(
                            "p (h x) -> p h x", x=bw)[:, :, 64:bw],
                        in_=vones[:, :, 64:bw])
                    tok0 = TT_COL[tt]
                    rows = 128 if tok0 + 128 <= COLS else COLS - tok0
                    for o0, on in ((0, 512), (512, 256)):
                        ps = psQ.tile([128, 512], F32, tag="psQ")
                        for kc in range(KC):
                            nc.tensor.matmul(
                                out=ps[:rows, 0:on],
                                lhsT=hsb[kc][:, tok0:tok0 + rows],
                                rhs=wsb[kc][:, o0:o0 + on],
                                start=(kc == 0),
                                stop=(kc == KC - 1 and not with_bias),
                            )
                        if with_bias:
                            nc.tensor.matmul(
                                out=ps[:rows, 0:on],
                                lhsT=bsb[0:1, 0:rows],
                                rhs=bsb[1 + bias_idx:2 + bias_idx, o0:o0 + on],
                                start=False, stop=True,
                            )
                        nc.vector.tensor_copy(
                            out=dst[:rows, ti, :].rearrange(
                                "p (h x) -> p h x", x=bw)[:, o0 // 64:(o0 + on) // 64, 0:64],
                            in_=ps[:rows, 0:on].rearrange("p (h x) -> p h x", x=64))

            # ---- global-row projections first ----
            proj_nat(vgN, wvg_t, (2, 3, 4, 5), 4, VW, vones_g)
            # qg projected directly transposed into zero-padded per-head
            # form; all six feature chunks accumulate into one PSUM tile
            psq = psQ.tile([128, KC * G], F32, tag="psQ", name="psq")
            for oc in range(KC):
                for kc in range(KC):
                    nc.tensor.matmul(
                        out=psq[:, oc * G:(oc + 1) * G],
                        lhsT=wqg_t[kc][:, oc * 128:(oc + 1) * 128],
                        rhs=hsb[kc][:, 1024:1088],
                        start=(kc == 0), stop=(kc == KC - 1 and not with_bias))
                if with_bias:
                    nc.tensor.matmul(
                        out=psq[:, oc * G:(oc + 1) * G],
                        lhsT=bsb[6:7, oc * 128:(oc + 1) * 128],
                        rhs=bsb[0:1, 0:G], start=False, stop=True)
            for oc in range(KC):
                nc.vector.tensor_copy(out=qgT[0:64, 2 * oc, :],
                                      in_=psq[0:64, oc * G:(oc + 1) * G])
                nc.vector.tensor_copy(out=qgT[64:128, 2 * oc + 1, :],
                                      in_=psq[64:128, oc * G:(oc + 1) * G])

            partial = dram.tile([G, NH * VW], F32)

            def glob_head(h):
                # transposed scores [keys, 64]; full-128 contraction via the
                # zero-padded qgT chunk
                pc = h // 2
                psg = psQ.tile([128, 4 * G], F32, tag="psQ",
                               name=f"psg{h}")
                for kt in range(4):
                    nc.tensor.matmul(
                        out=psg[:, kt * G:(kt + 1) * G],
                        lhsT=kgT[:, pc, kt * 128:(kt + 1) * 128],
                        rhs=qgT[:, h, :],
                        start=True, stop=True)
                pgt = w2.tile([128, 4 * G], F16, tag="pgt", name=f"pgt{h}")
                nc.scalar.activation(out=pgt, in_=psg, func=Exp, scale=SCALE)
                pspv = psO.tile([G, VW], F32, tag="psO", name=f"pspv{h}")
                for kt in range(4):
                    nc.tensor.matmul(out=pspv,
                                     lhsT=pgt[:, kt * G:(kt + 1) * G],
                                     rhs=vgN[:, kt, VW * h:VW * h + VW],
                                     start=(kt == 0), stop=(kt == 3))
                part = w2.tile([G, VW], F32, tag="part", name=f"part{h}")
                nc.vector.tensor_copy(out=part, in_=pspv)
                next_eng().dma_start(out=partial[:, h * VW:(h + 1) * VW],
                                     in_=part)

            # glob heads dovetail into the kgT projection's PE bubbles with a
            # one-chunk pipeline offset so chunk oc's matmuls cover chunk
            # oc-1's evacuation latency
            proj_T(kgT, wkg_t, ((0, 512),), 3, 0,
                   hook=lambda oc: None if oc == 0 else (
                       glob_head(2 * (oc - 1)), glob_head(2 * oc - 1)))
            glob_head(10)
            glob_head(11)
            # AllReduce the partials; the host does the final division. The
            # DRAM->DRAM copy into the output rides the gpsimd queue, which
            # nothing latency-critical shares.
            reduced = dram.tile([G, NH * VW], F32)
            nc.gpsimd.collective_compute(
                "AllReduce", mybir.AluOpType.add,
                replica_groups=[list(range(NCORE))],
                ins=[partial.opt()], outs=[reduced.opt()])
            nc.gpsimd.dma_start(out=d_outgraw[:], in_=reduced)

            # ---- banded projections ----
            proj_T(qT, wq_t, ((0, 512),), 0, 0, split=True)
            proj_T(kT, wk_t, ((0, 512), (512, 512), (1024, 64)), 1, 0)
            proj_nat(vE, wv_t, (0, 1, 2, 3, 4, 5, 6, 7, 8), 2, EW, vones_e)

            # ---- banded + global-column attention (the bulk) ----
            osb = late.tile([128, 4, H], F16, tag="osb")
            for h in range(NH):
                pc = h // 2
                pso = psO.tile([128, TPC], F32, tag="psO")
                exs = []
                for pa, pb in JX_PAIRS:
                    wa = JX_T[pa][1] - JX_T[pa][0]
                    wb = JX_T[pb][1] - JX_T[pb][0]
                    pss = psQ.tile([128, 1024], F32, tag="psQ")
                    ex = work.tile([128, 1024], F16, tag="ex", bufs=3)
                    exs.append(ex)
                    for jx, off in ((pa, 0), (pb, wa)):
                        t0, tn = JX_T[jx]
                        nc.tensor.matmul(
                            out=pss[:, off:off + tn - t0],
                            lhsT=kT[:, pc, JX_COL[jx]:JX_COL[jx] + 128],
                            rhs=qT[:, h, t0:tn],
                            start=True, stop=True)
                    nc.scalar.activation(out=ex[:, 0:wa + wb],
                                         in_=pss[:, 0:wa + wb],
                                         func=Exp, scale=SCALE)
                for pi, (c0, c1), (k0, k1), eng in MASK_OPS:
                    mul = nc.vector.tensor_mul if eng == "v" \
                        else nc.gpsimd.tensor_mul
                    mul(exs[pi][:, c0:c1], exs[pi][:, c0:c1], msb[:, k0:k1])
                first_pv = True
                for (pa, pb), ex in zip(JX_PAIRS, exs):
                    wa = JX_T[pa][1] - JX_T[pa][0]
                    for jx, off in ((pa, 0), (pb, wa)):
                        t0, tn = JX_T[jx]
                        nc.tensor.matmul(
                            out=pso[:, t0:tn],
                            lhsT=vE[:, jx, EW * h:EW * (h + 1)],
                            rhs=ex[:, off:off + tn - t0],
                            start=first_pv, stop=False)
                        first_pv = False
                # global-key columns, joint softmax
                pss = psQ.tile([128, 1024], F32, tag="psQ")
                nc.tensor.matmul(
                    out=pss[0:G, 0:TPC], lhsT=kT[:, pc, 1024:1088],
                    rhs=qT[:, h, :],
                    start=True, stop=True)
                exg = exg2[h % 2]
                nc.scalar.activation(out=exg[0:G, :], in_=pss[0:G, 0:TPC],
                                     func=Exp, scale=SCALE)
                nc.tensor.matmul(
                    out=pso, lhsT=vE[:, 8, EW * h:EW * (h + 1)],
                    rhs=exg, start=False, stop=True)
                ot = w2.tile([VW, TPC], F16, tag="ot")
                nc.vector.tensor_copy(out=ot, in_=pso[0:VW, :])
                # 4 transposes into one psum tile, merged reciprocal
                pstr = psT.tile([128, 4 * VW], F16, tag="psT")
                for tt in range(4):
                    nc.tensor.transpose(pstr[:, tt * VW:(tt + 1) * VW],
                                        ot[:, tt * 128:(tt + 1) * 128],
                                        ident[0:VW, 0:VW])
                rec = work.tile([128, 4], F32, tag="rec", bufs=4)
                nc.vector.reciprocal(
                    out=rec,
                    in_=pstr.rearrange("p (tt x) -> p tt x", x=VW)[:, :, 64])
                for tt in range(4):
                    nc.vector.tensor_scalar_mul(
                        osb[:, tt, 64 * h:64 * (h + 1)],
                        in0=pstr[:, tt * VW:tt * VW + 64],
                        scalar1=rec[:, tt:tt + 1])
            for i4 in range(4):
                next_eng().dma_start(out=d_out[128 * i4:128 * (i4 + 1), :],
                                     in_=osb[:, i4, :])

    nc.compile()
    return nc


def _pm(x):
    # [KC*128, X] row-major -> partition-major [128, KC*X] fp16
    X = x.shape[1]
    return np.ascontiguousarray(
        x.reshape(KC, 128, X).transpose(1, 0, 2).reshape(128, KC * X)
    ).astype(np.float16)


def _host_inputs(hs, weights, biases):
    """Build the 8 per-core input maps from full inputs."""
    hsT = np.ascontiguousarray(hs.T)               # [H, S]
    ident = np.eye(128, dtype=np.float16)
    vimg_e = np.zeros((128, NH * EW), np.float16)
    vimg_e[:, 64::EW] = 1.0
    vimg_g = np.zeros((128, NH * VW), np.float16)
    vimg_g[:, 64::VW] = 1.0
    consts = np.concatenate([ident, vimg_e, vimg_g], axis=1)

    with_bias = any(np.any(b) for b in biases)
    if with_bias:
        brow = np.zeros((7, COLS), np.float16)
        brow[0, :] = 1.0
        for i, b in enumerate(biases):
            brow[1 + i, :H] = b
    pp = np.arange(128)[:, None]                    # key pos within jx tile
    ii = np.arange(C)[None, :]                      # query pos within chunk
    in_maps = []
    for core in range(NCORE):
        hst = np.zeros((H, COLS), np.float32)
        lo = TPC * core - C
        hi = TPC * core + TPC + C
        clo, chi = max(lo, 0), min(hi, S)
        hst[:, clo - lo:chi - lo] = hsT[:, clo:chi]
        hst[:, EXT:] = hsT[:, :G]
        # reorder ext columns to the stored [own | haloL | haloR | glob]
        # layout (see JX_COL/TT_COL)
        hst = np.concatenate([hst[:, C:C + TPC], hst[:, 0:C],
                              hst[:, C + TPC:EXT], hst[:, EXT:]], axis=1)
        # masks packed per jx at JX_PACK offsets, covering JX_MASK regions
        mk = np.ones((128, 2304), np.float16)
        for jx in range(8):
            m0, m1 = JX_MASK[jx]
            mo = JX_PACK[jx]
            for cl in range(2):
                jt = jx - 2 * cl
                if not 0 <= jt <= 5:
                    continue
                t_lo = C * cl
                if t_lo < m0 or t_lo >= m1:
                    continue
                n = 2 * core + cl
                jj = 128 * jt + pp                  # strip pos within chunk
                ka = n * C - C + jj                 # absolute key pos
                valid = ((jj >= ii) & (jj <= ii + 2 * C)
                         & (ka >= G) & (ka < S))
                mk[:, mo + t_lo - m0:mo + t_lo - m0 + C] = valid
        im = {
            "hsT": _pm(hst),
            "wq": _pm(weights[0]), "wk": _pm(weights[1]),
            "wv": _pm(weights[2]), "wkg": _pm(weights[3]),
            "wvg": _pm(weights[4]), "wqg": _pm(weights[5]),
            "masks": mk,
            "consts": consts,
        }
        if with_bias:
            im["biasrow"] = brow
        in_maps.append(im)
    return in_maps, with_bias


def kernel(hidden_states, Wq, bq, Wk, bk, Wv, bv, Wqg, bqg, Wkg, bkg,
           Wvg, bvg):
    hs = np.asarray(hidden_states, np.float32).reshape(S, H)
    weights = [np.ascontiguousarray(np.asarray(w, np.float32))
               for w in (Wq, Wk, Wv, Wkg, Wvg, Wqg)]
    biases = [np.asarray(b, np.float32)
              for b in (bq, bk, bv, bkg, bvg, bqg)]
    in_maps, with_bias = _host_inputs(hs, weights, biases)

    if with_bias not in _PROG_CACHE:
        _PROG_CACHE[with_bias] = _build_program(with_bias)
    nc = _PROG_CACHE[with_bias]

    res = run_bass_kernel_spmd(nc, in_maps, list(range(NCORE)))

    out = np.empty((S, H), np.float32)
    for core in range(NCORE):
        out[TPC * core:TPC * (core + 1)] = res.results[core]["out"]
    # host-side normalization of the AllReduced global-row partials
    raw = np.asarray(res.results[0]["outgraw"],
                     np.float32).reshape(G, NH, VW)
    out[:G] = (raw[:, :, :64] / raw[:, :, 64:65]).reshape(G, H)
    return out.reshape(1, S, H)


# revision 41
# speedup vs baseline: 1.0494x; 1.0494x over previous
"""Longformer sliding-window + global attention layer on 8 Trainium2 NeuronCores.

Sharding: sequence-parallel over the 4096 tokens (512 per core, all 12 heads).
Each core recomputes the k/v halo (256 tokens each side) and the 64 global
k/v tokens locally from zero-padded hsT input, so the program is uniform SPMD.
The global-query rows (first 64 tokens attend to everything) are computed as
flash-style partial sums over each core's 512 tokens, AllReduced directly into
an output DRAM buffer, and normalized on the host — no on-device consumer of
the collective exists, so it can never stall the banded pipeline.

v3 layout strategy (all matmuls fp16, PSUM accumulate f32). The PE runs
~0.55 ns/row only when both the contraction and output-partition dims are a
full 128; 64-wide geometry costs ~0.9-1.05 ns/row. So:
  - qT / qgT are stored zero-padded per head (12 chunks of [128, t]: head h
    occupies feature rows 64*(h%2)..+64, zeros elsewhere) so every score
    matmul contracts over the full 128 partitions of the shared k chunk —
    the other head's k columns are killed by the zeros.
  - vE uses 128-wide head blocks (64 v | ones | 63 zeros) so PV matmuls have
    128 output partitions; the softmax denominator lands in PSUM partition 64
    via the ones column. exg (global-column probs) is kept in persistent
    [128, 512] tiles whose lower 64 rows are zeroed once, so the global-column
    PV also contracts over 128.
  - Banded attention uses transposed scores over the core's 8 extended-window
    key tiles (jx) in pairs sharing a 2-bank PSUM tile (one exp per pair);
    0/1 mask multiplies cover only band/col boundary regions. A PE transpose
    + per-partition reciprocal scale normalizes [t, d] tiles.
"""
import numpy as np

import concourse.bacc as bacc
import concourse.mybir as mybir
import concourse.tile as tile
from concourse.bass_utils import run_bass_kernel_spmd

F32 = mybir.dt.float32
F16 = mybir.dt.float16
Exp = mybir.ActivationFunctionType.Exp

S, H, NH, HD = 4096, 768, 12, 64
C = 256               # chunk / one-sided window
G = 64                # global tokens
NCORE = 8
TPC = S // NCORE      # 512 tokens per core
EXT = TPC + 2 * C     # 1024 ext window
COLS = EXT + G        # 1088 = ext | glob
KC = H // 128         # 6 hidden chunks
VW = 66               # per-head vgN block: 64 v | ones | pad
EW = 128              # per-head vE block: 64 v | ones | 63 zeros
SCALE = 1.0 / 8.0     # 1/sqrt(HD)

# per key-tile jx: (t0, tn) query range its chunk windows cover, and the
# t-range needing the 0/1 mask multiply
JX_T = {0: (0, 256), 1: (0, 256), 2: (0, 512), 3: (0, 512),
        4: (0, 512), 5: (0, 512), 6: (256, 512), 7: (256, 512)}
JX_MASK = {0: (0, 256), 1: (0, 256), 2: (0, 512), 3: (256, 512),
           4: (0, 256), 5: (0, 256), 6: (256, 512), 7: (256, 512)}
# pairs share one [128, 1024] PSUM tile; (2,3) first so PV start=True is full
JX_PAIRS = [(2, 3), (4, 5), (0, 1), (6, 7)]
# packed col offset of each jx's mask region in the [128, 2304] masks input
JX_PACK = {2: 0, 3: 512, 4: 768, 5: 1024, 0: 1280, 1: 1536, 6: 1792, 7: 2048}
# mask ops: (pair_index, ex-tile col range, packed range, engine)
MASK_OPS = [
    (0, (0, 512), (0, 512), "v"),         # jx2
    (0, (768, 1024), (512, 768), "v"),    # jx3
    (1, (0, 256), (768, 1024), "v"),      # jx4
    (1, (512, 768), (1024, 1280), "v"),   # jx5
    (2, (0, 512), (1280, 1792), "v"),     # jx0|jx1 contiguous
    (3, (0, 512), (1792, 2304), "v"),     # jx6|jx7 contiguous
]

_PROG_CACHE = {}


def _build_program(with_bias: bool):
    nc = bacc.Bacc("TRN2", target_bir_lowering=False, debug=False,
                   num_devices=NCORE)
    d_hsT = nc.declare_dram_parameter("hsT", [128, KC * COLS], F16,
                                      isOutput=False)
    d_w = {
        n: nc.declare_dram_parameter(n, [128, KC * H], F16, isOutput=False)
        for n in ("wq", "wk", "wv", "wkg", "wvg", "wqg")
    }
    d_masks = nc.declare_dram_parameter("masks", [128, 2304], F16,
                                        isOutput=False)
    CW = 128 + NH * EW + NH * VW    # ident | vE ones image | vgN ones image
    d_consts = nc.declare_dram_parameter("consts", [128, CW], F16,
                                         isOutput=False)
    if with_bias:
        d_brow = nc.declare_dram_parameter("biasrow", [7, COLS], F16,
                                           isOutput=False)
    d_out = nc.declare_dram_parameter("out", [TPC, H], F16, isOutput=True)
    d_outgraw = nc.declare_dram_parameter("outgraw", [G, NH * VW], F32,
                                          isOutput=True)

    with tile.TileContext(nc) as tc:
        with (
            tc.tile_pool(name="const", bufs=1) as const,
            tc.tile_pool(name="work", bufs=2) as work,
            tc.tile_pool(name="w2", bufs=3) as w2,
            tc.tile_pool(name="late", bufs=1) as late,
            tc.tile_pool(name="dram", bufs=2, space="DRAM") as dram,
            tc.tile_pool(name="psQ", bufs=2, space="PSUM") as psQ,
            tc.tile_pool(name="psO", bufs=2, space="PSUM") as psO,
            tc.tile_pool(name="psT", bufs=2, space="PSUM") as psT,
        ):
            engs = [nc.sync, nc.scalar, nc.gpsimd]
            eng_i = [0]

            def next_eng():
                e = engs[eng_i[0] % 3]
                eng_i[0] += 1
                return e

            # ---- resident loads, chunked + ordered by first consumer ----
            hsb_t = late.tile([128, KC, COLS], F16, tag="ph")
            for kc in range(KC):
                next_eng().dma_start(out=hsb_t[:, kc, :],
                                     in_=d_hsT[:, kc * COLS:(kc + 1) * COLS])
            hsb = [hsb_t[:, kc, :] for kc in range(KC)]

            def load_w(name):
                t = const.tile([128, KC, H], F16, tag=f"w_{name}",
                               name=f"w_{name}")
                for kc in range(KC):
                    next_eng().dma_start(
                        out=t[:, kc, :], in_=d_w[name][:, kc * H:(kc + 1) * H])
                return [t[:, kc, :] for kc in range(KC)]

            wvg_t = load_w("wvg")
            csb = const.tile([128, CW], F16)
            next_eng().dma_start(out=csb, in_=d_consts[:])
            ident = csb[:, 0:128]
            # per EW-block: col 64 = 1.0 (ones col), rest 0
            vones_e = csb[:, 128:128 + NH * EW].rearrange(
                "p (h x) -> p h x", x=EW)
            # per VW-block: col 64 = 1.0, col 65 = 0
            vones_g = csb[:, 128 + NH * EW:CW].rearrange(
                "p (h x) -> p h x", x=VW)
            if with_bias:
                bsb = const.tile([7, COLS], F16)
                next_eng().dma_start(out=bsb, in_=d_brow[:])
            wqg_t = load_w("wqg")
            wkg_t = load_w("wkg")
            wq_t = load_w("wq")
            msb = const.tile([128, 2304], F16)
            next_eng().dma_start(out=msb, in_=d_masks[:])
            wk_t = load_w("wk")
            wv_t = load_w("wv")

            kT = const.tile([128, KC, COLS], F16)    # [o, t] all heads
            qT = const.tile([128, NH, TPC], F16)     # zero-padded per head
            vE = const.tile([128, 9, NH * EW], F16)  # natural v + ones cols
            kgT = const.tile([128, KC, TPC], F16)
            vgN = const.tile([128, 4, NH * VW], F16)
            qgT = const.tile([128, NH, G], F16)      # zero-padded per head
            exg2 = [const.tile([128, TPC], F16, tag=f"exg{i}",
                               name=f"exg{i}")
                    for i in range(2)]
            # zero the pad regions once, while the PE waits on input DMA
            nc.vector.memset(qT[:, :, :], 0.0)
            nc.vector.memset(qgT[:, :, :], 0.0)
            nc.vector.memset(vE[64:128, 8, :], 0.0)
            nc.vector.memset(exg2[0][:, :], 0.0)
            nc.vector.memset(exg2[1][:, :], 0.0)

            def proj_T(dst, wsb, segs, bias_idx, dst_off, split=False,
                       hook=None):
                # dst[o, t] = W.T @ hsT cols; split=True scatters the two
                # 64-row head halves into zero-padded per-head chunks
                for oc in range(KC):
                    for c0, cn in segs:
                        ps = psQ.tile([128, 512], F32, tag="psQ")
                        for kc in range(KC):
                            nc.tensor.matmul(
                                out=ps[:, 0:cn],
                                lhsT=wsb[kc][:, oc * 128:(oc + 1) * 128],
                                rhs=hsb[kc][:, c0:c0 + cn],
                                start=(kc == 0),
                                stop=(kc == KC - 1 and not with_bias),
                            )
                        if with_bias:
                            nc.tensor.matmul(
                                out=ps[:, 0:cn],
                                lhsT=bsb[1 + bias_idx:2 + bias_idx,
                                         oc * 128:(oc + 1) * 128],
                                rhs=bsb[0:1, 0:cn],
                                start=False, stop=True,
                            )
                        d0 = c0 - dst_off
                        if split:
                            nc.vector.tensor_copy(
                                out=dst[0:64, 2 * oc, d0:d0 + cn],
                                in_=ps[0:64, 0:cn])
                            nc.vector.tensor_copy(
                                out=dst[64:128, 2 * oc + 1, d0:d0 + cn],
                                in_=ps[64:128, 0:cn])
                        else:
                            nc.vector.tensor_copy(
                                out=dst[:, oc, d0:d0 + cn], in_=ps[:, 0:cn])
                    if hook is not None:
                        hook(oc)

            def proj_nat(dst, wsb, tts, bias_idx, bw, vones):
                # dst[t, head-block] with bw-stride head blocks
                for ti, tt in enumerate(tts):
                    # only the ones|pad columns of each block need init;
                    # value cols are overwritten by the PSUM evacuation
                    nc.vector.tensor_copy(
                        out=dst[:, ti, :].rearrange(
                            "p (h x) -> p h x", x=bw)[:, :, 64:bw],
                        in_=vones[:, :, 64:bw])
                    tok0 = tt * 128
                    rows = 128 if tok0 + 128 <= COLS else COLS - tok0
                    for o0, on in ((0, 512), (512, 256)):
                        ps = psQ.tile([128, 512], F32, tag="psQ")
                        for kc in range(KC):
                            nc.tensor.matmul(
                                out=ps[:rows, 0:on],
                                lhsT=hsb[kc][:, tok0:tok0 + rows],
                                rhs=wsb[kc][:, o0:o0 + on],
                                start=(kc == 0),
                                stop=(kc == KC - 1 and not with_bias),
                            )
                        if with_bias:
                            nc.tensor.matmul(
                                out=ps[:rows, 0:on],
                                lhsT=bsb[0:1, 0:rows],
                                rhs=bsb[1 + bias_idx:2 + bias_idx, o0:o0 + on],
                                start=False, stop=True,
                            )
                        nc.vector.tensor_copy(
                            out=dst[:rows, ti, :].rearrange(
                                "p (h x) -> p h x", x=bw)[:, o0 // 64:(o0 + on) // 64, 0:64],
                            in_=ps[:rows, 0:on].rearrange("p (h x) -> p h x", x=64))

            # ---- global-row projections first ----
            proj_nat(vgN, wvg_t, (2, 3, 4, 5), 4, VW, vones_g)
            # qg projected directly transposed into zero-padded per-head
            # form; all six feature chunks accumulate into one PSUM tile
            psq = psQ.tile([128, KC * G], F32, tag="psQ", name="psq")
            for oc in range(KC):
                for kc in range(KC):
                    nc.tensor.matmul(
                        out=psq[:, oc * G:(oc + 1) * G],
                        lhsT=wqg_t[kc][:, oc * 128:(oc + 1) * 128],
                        rhs=hsb[kc][:, EXT:EXT + G],
                        start=(kc == 0), stop=(kc == KC - 1 and not with_bias))
                if with_bias:
                    nc.tensor.matmul(
                        out=psq[:, oc * G:(oc + 1) * G],
                        lhsT=bsb[6:7, oc * 128:(oc + 1) * 128],
                        rhs=bsb[0:1, 0:G], start=False, stop=True)
            for oc in range(KC):
                nc.vector.tensor_copy(out=qgT[0:64, 2 * oc, :],
                                      in_=psq[0:64, oc * G:(oc + 1) * G])
                nc.vector.tensor_copy(out=qgT[64:128, 2 * oc + 1, :],
                                      in_=psq[64:128, oc * G:(oc + 1) * G])

            partial = dram.tile([G, NH * VW], F32)

            def glob_head(h):
                # transposed scores [keys, 64]; full-128 contraction via the
                # zero-padded qgT chunk
                pc = h // 2
                psg = psQ.tile([128, 4 * G], F32, tag="psQ",
                               name=f"psg{h}")
                for kt in range(4):
                    nc.tensor.matmul(
                        out=psg[:, kt * G:(kt + 1) * G],
                        lhsT=kgT[:, pc, kt * 128:(kt + 1) * 128],
                        rhs=qgT[:, h, :],
                        start=True, stop=True)
                pgt = w2.tile([128, 4 * G], F16, tag="pgt", name=f"pgt{h}")
                nc.scalar.activation(out=pgt, in_=psg, func=Exp, scale=SCALE)
                pspv = psO.tile([G, VW], F32, tag="psO", name=f"pspv{h}")
                for kt in range(4):
                    nc.tensor.matmul(out=pspv,
                                     lhsT=pgt[:, kt * G:(kt + 1) * G],
                                     rhs=vgN[:, kt, VW * h:VW * h + VW],
                                     start=(kt == 0), stop=(kt == 3))
                part = w2.tile([G, VW], F32, tag="part", name=f"part{h}")
                nc.vector.tensor_copy(out=part, in_=pspv)
                next_eng().dma_start(out=partial[:, h * VW:(h + 1) * VW],
                                     in_=part)

            # glob heads dovetail into the kgT projection's PE bubbles with a
            # one-chunk pipeline offset so chunk oc's matmuls cover chunk
            # oc-1's evacuation latency
            proj_T(kgT, wkg_t, ((C, 512),), 3, C,
                   hook=lambda oc: None if oc == 0 else (
                       glob_head(2 * (oc - 1)), glob_head(2 * oc - 1)))
            glob_head(10)
            glob_head(11)
            # AllReduce the partials; the host does the final division. The
            # DRAM->DRAM copy into the output rides the gpsimd queue, which
            # nothing latency-critical shares.
            reduced = dram.tile([G, NH * VW], F32)
            nc.gpsimd.collective_compute(
                "AllReduce", mybir.AluOpType.add,
                replica_groups=[list(range(NCORE))],
                ins=[partial.opt()], outs=[reduced.opt()])
            nc.gpsimd.dma_start(out=d_outgraw[:], in_=reduced)

            # ---- banded projections ----
            proj_T(qT, wq_t, ((C, 512),), 0, C, split=True)
            proj_T(kT, wk_t, ((0, 512), (512, 320), (832, 256)), 1, 0)
            proj_nat(vE, wv_t, (0, 1, 2, 3, 4, 5, 6, 7, 8), 2, EW, vones_e)

            # ---- banded + global-column attention (the bulk) ----
            osb = late.tile([128, 4, H], F16, tag="osb")
            for h in range(NH):
                pc = h // 2
                pso = psO.tile([128, TPC], F32, tag="psO")
                exs = []
                for pa, pb in JX_PAIRS:
                    wa = JX_T[pa][1] - JX_T[pa][0]
                    wb = JX_T[pb][1] - JX_T[pb][0]
                    pss = psQ.tile([128, 1024], F32, tag="psQ")
                    ex = work.tile([128, 1024], F16, tag="ex", bufs=3)
                    exs.append(ex)
                    for jx, off in ((pa, 0), (pb, wa)):
                        t0, tn = JX_T[jx]
                        nc.tensor.matmul(
                            out=pss[:, off:off + tn - t0],
                            lhsT=kT[:, pc, 128 * jx:128 * (jx + 1)],
                            rhs=qT[:, h, t0:tn],
                            start=True, stop=True)
                    nc.scalar.activation(out=ex[:, 0:wa + wb],
                                         in_=pss[:, 0:wa + wb],
                                         func=Exp, scale=SCALE)
                for pi, (c0, c1), (k0, k1), eng in MASK_OPS:
                    mul = nc.vector.tensor_mul if eng == "v" \
                        else nc.gpsimd.tensor_mul
                    mul(exs[pi][:, c0:c1], exs[pi][:, c0:c1], msb[:, k0:k1])
                first_pv = True
                for (pa, pb), ex in zip(JX_PAIRS, exs):
                    wa = JX_T[pa][1] - JX_T[pa][0]
                    for jx, off in ((pa, 0), (pb, wa)):
                        t0, tn = JX_T[jx]
                        nc.tensor.matmul(
                            out=pso[:, t0:tn],
                            lhsT=vE[:, jx, EW * h:EW * (h + 1)],
                            rhs=ex[:, off:off + tn - t0],
                            start=first_pv, stop=False)
                        first_pv = False
                # global-key columns, joint softmax
                pss = psQ.tile([128, 1024], F32, tag="psQ")
                nc.tensor.matmul(
                    out=pss[0:G, 0:TPC], lhsT=kT[:, pc, EXT:EXT + G],
                    rhs=qT[:, h, :],
                    start=True, stop=True)
                exg = exg2[h % 2]
                nc.scalar.activation(out=exg[0:G, :], in_=pss[0:G, 0:TPC],
                                     func=Exp, scale=SCALE)
                nc.tensor.matmul(
                    out=pso, lhsT=vE[:, 8, EW * h:EW * (h + 1)],
                    rhs=exg, start=False, stop=True)
                ot = w2.tile([VW, TPC], F16, tag="ot")
                nc.vector.tensor_copy(out=ot, in_=pso[0:VW, :])
                # 4 transposes into one psum tile, merged reciprocal
                pstr = psT.tile([128, 4 * VW], F16, tag="psT")
                for tt in range(4):
                    nc.tensor.transpose(pstr[:, tt * VW:(tt + 1) * VW],
                                        ot[:, tt * 128:(tt + 1) * 128],
                                        ident[0:VW, 0:VW])
                rec = work.tile([128, 4], F32, tag="rec", bufs=4)
                nc.vector.reciprocal(
                    out=rec,
                    in_=pstr.rearrange("p (tt x) -> p tt x", x=VW)[:, :, 64])
                for tt in range(4):
                    nc.vector.tensor_scalar_mul(
                        osb[:, tt, 64 * h:64 * (h + 1)],
                        in0=pstr[:, tt * VW:tt * VW + 64],
                        scalar1=rec[:, tt:tt + 1])
            for i4 in range(4):
                next_eng().dma_start(out=d_out[128 * i4:128 * (i4 + 1), :],
                                     in_=osb[:, i4, :])

    nc.compile()
    return nc


def _pm(x):
    # [KC*128, X] row-major -> partition-major [128, KC*X] fp16
    X = x.shape[1]
    return np.ascontiguousarray(
        x.reshape(KC, 128, X).transpose(1, 0, 2).reshape(128, KC * X)
    ).astype(np.float16)


def _host_inputs(hs, weights, biases):
    """Build the 8 per-core input maps from full inputs."""
    hsT = np.ascontiguousarray(hs.T)               # [H, S]
    ident = np.eye(128, dtype=np.float16)
    vimg_e = np.zeros((128, NH * EW), np.float16)
    vimg_e[:, 64::EW] = 1.0
    vimg_g = np.zeros((128, NH * VW), np.float16)
    vimg_g[:, 64::VW] = 1.0
    consts = np.concatenate([ident, vimg_e, vimg_g], axis=1)

    with_bias = any(np.any(b) for b in biases)
    if with_bias:
        brow = np.zeros((7, COLS), np.float16)
        brow[0, :] = 1.0
        for i, b in enumerate(biases):
            brow[1 + i, :H] = b
    pp = np.arange(128)[:, None]                    # key pos within jx tile
    ii = np.arange(C)[None, :]                      # query pos within chunk
    in_maps = []
    for core in range(NCORE):
        hst = np.zeros((H, COLS), np.float32)
        lo = TPC * core - C
        hi = TPC * core + TPC + C
        clo, chi = max(lo, 0), min(hi, S)
        hst[:, clo - lo:chi - lo] = hsT[:, clo:chi]
        hst[:, EXT:] = hsT[:, :G]
        # masks packed per jx at JX_PACK offsets, covering JX_MASK regions
        mk = np.ones((128, 2304), np.float16)
        for jx in range(8):
            m0, m1 = JX_MASK[jx]
            mo = JX_PACK[jx]
            for cl in range(2):
                jt = jx - 2 * cl
                if not 0 <= jt <= 5:
                    continue
                t_lo = C * cl
                if t_lo < m0 or t_lo >= m1:
                    continue
                n = 2 * core + cl
                jj = 128 * jt + pp                  # strip pos within chunk
                ka = n * C - C + jj                 # absolute key pos
                valid = ((jj >= ii) & (jj <= ii + 2 * C)
                         & (ka >= G) & (ka < S))
                mk[:, mo + t_lo - m0:mo + t_lo - m0 + C] = valid
        im = {
            "hsT": _pm(hst),
            "wq": _pm(weights[0]), "wk": _pm(weights[1]),
            "wv": _pm(weights[2]), "wkg": _pm(weights[3]),
            "wvg": _pm(weights[4]), "wqg": _pm(weights[5]),
            "masks": mk,
            "consts": consts,
        }
        if with_bias:
            im["biasrow"] = brow
        in_maps.append(im)
    return in_maps, with_bias


def kernel(hidden_states, Wq, bq, Wk, bk, Wv, bv, Wqg, bqg, Wkg, bkg,
           Wvg, bvg):
    hs = np.asarray(hidden_states, np.float32).reshape(S, H)
    weights = [np.ascontiguousarray(np.asarray(w, np.float32))
               for w in (Wq, Wk, Wv, Wkg, Wvg, Wqg)]
    biases = [np.asarray(b, np.float32)
              for b in (bq, bk, bv, bkg, bvg, bqg)]
    in_maps, with_bias = _host_inputs(hs, weights, biases)

    if with_bias not in _PROG_CACHE:
        _PROG_CACHE[with_bias] = _build_program(with_bias)
    nc = _PROG_CACHE[with_bias]

    res = run_bass_kernel_spmd(nc, in_maps, list(range(NCORE)))

    out = np.empty((S, H), np.float32)
    for core in range(NCORE):
        out[TPC * core:TPC * (core + 1)] = res.results[core]["out"]
    # host-side normalization of the AllReduced global-row partials
    raw = np.asarray(res.results[0]["outgraw"],
                     np.float32).reshape(G, NH, VW)
    out[:G] = (raw[:, :, :64] / raw[:, :, 64:65]).reshape(G, H)
    return out.reshape(1, S, H)
